# revision 6
# baseline (speedup 1.0000x reference)
"""Contrastive loss (batch-hard triplet, within batch) on 8 Trainium2 cores.

Math (matches the jax reference):
    xn = x / ||x||_2 (rows)                      [B, C] = [4096, 1024]
    g[i,j] = xn_i . xn_j
    d[i,j] = max(2 - 2 g, 0)   (since ||xn||=1)
    pos_i  = sum_{j: same label, j != i} d[i,j]
    neg_i  = min_{j: diff label} d[i,j]
    loss   = mean(relu(pos_i + 0.5 - neg_i))

Sharding: rows (anchors) split 512/core; every core computes its
[512, 4096] tile of the distance matrix. Host passes xT (bf16 layout
staging) rotated per-core so each core's anchor columns come FIRST —
the anchor operand is then just a view of j-slice 0 (no separate
anchor pass). ALL math (norms, normalize, matmul, reductions) runs on
device.

Device pipeline per j-slice s (8 slices of 512 columns):
  DMA xT chunks [128,512] bf16 -> square -> fp8 pairs -> PE fp8
  DoubleRow ones-matmul column-reduce -> sq [1,512] -> ACT
  sqrt(sq/256) -> bf16 -> PE broadcast [128,512] -> DVE
  reciprocal_approx_fast -> invb = 16/||x|| -> multiply (DVE/Pool)
  -> xnt[s] fp8 (k-major [128, 8, 512])

Label-mask fusion: 64 one-hot rows scaled +128 (anchor) / -128 (rhs)
append to the contraction, so PSUM = 256*(g - 64*same). Per tile:
    pos_half = sum_j relu(-PSUM/256 - 63)   (ACT, accum_out)
    mx       = max_j PSUM                   (DVE/Pool reduce)
    loss_i   = relu(2*(pos_half - relu(1 - mx/256)) + 0.5)

Main matmuls are fp8 DoubleRow (2 contraction rows/cycle), slices
processed in pairs with shared lhsT so weight loads amortize, and all
prep PE-work for slice pair p+1 is issued ahead of pair p's main
matmuls so the PE never idles (keeps the HAM clock-gate at 2.4 GHz).
Per-core output is sum(loss_i)/4096; the host adds the 8 partials.
"""

import sys

if "/opt/trn_rl_repo" not in sys.path:
    sys.path.insert(0, "/opt/trn_rl_repo")

from contextlib import ExitStack

import ml_dtypes
import numpy as np

import concourse.bass as bass
import concourse.tile as tile
from concourse import bacc, mybir
from concourse.bass_utils import run_bass_kernel_spmd

B = 4096          # batch rows
C = 1024          # features
NCORES = 8
BA = B // NCORES  # anchors per core = 512
P = 128
KC = C // P       # 8 feature chunks of 128
NB = 512          # j-slice width
NJ = B // NB      # 8 j slices (slice 0 doubles as the anchor slice)
NM = BA // P      # 4 anchor blocks (M=128 each)
NCP = KC // 2     # 4 DoubleRow chunk-pairs
NLAB = 64

F32 = mybir.dt.float32
BF16 = mybir.dt.bfloat16
FP8 = mybir.dt.float8e4
AF = mybir.ActivationFunctionType
AX = mybir.AxisListType
DR = mybir.MatmulPerfMode.DoubleRow

ALPHA = 128.0     # onehot scale; product = -64 * XSCALE^2
XSCALE = 16.0     # fp8 matmul operands are 16*xn
PSC = 1.0 / (XSCALE * XSCALE)   # PSUM -> m rescale


def build_kernel():
    nc = bacc.Bacc("TRN2", target_bir_lowering=False, debug=False,
                   num_devices=NCORES)
    xt_d = nc.dram_tensor("xT", (C, B), BF16, kind="ExternalInput").ap()
    ohp_d = nc.dram_tensor("ohp", (NLAB, BA), FP8, kind="ExternalInput").ap()
    ohn_d = nc.dram_tensor("ohn", (NLAB, B), FP8, kind="ExternalInput").ap()
    out_d = nc.dram_tensor("out", (1, 1), F32, kind="ExternalOutput").ap()

    with tile.TileContext(nc) as tc, ExitStack() as ctx:
        big = ctx.enter_context(tc.tile_pool(name="big", bufs=1))
        xload = ctx.enter_context(tc.tile_pool(name="xload", bufs=36))
        sqp = ctx.enter_context(tc.tile_pool(name="sqp", bufs=10))
        nrows = ctx.enter_context(tc.tile_pool(name="nrows", bufs=3))
        invp = ctx.enter_context(tc.tile_pool(name="invp", bufs=3))
        scratch = ctx.enter_context(tc.tile_pool(name="scratch", bufs=2))
        psum = ctx.enter_context(tc.tile_pool(name="psum", bufs=5, space="PSUM"))
        psum2 = ctx.enter_context(tc.tile_pool(name="psum2", bufs=1, space="PSUM"))
        small = ctx.enter_context(tc.tile_pool(name="small", bufs=1))

        # xnt[s][p, c, j] = fp8(XSCALE * xn[s*512 + j, c*128 + p])
        xnts = [big.tile([P, KC, NB], FP8, name=f"xnt{s}", tag=f"xnt{s}")
                for s in range(NJ)]
        ohp = big.tile([NLAB, BA], FP8)
        ohn = big.tile([NLAB, B], FP8)
        pos_all = big.tile([P, NM * NJ], F32)
        max_all = big.tile([P, NM * NJ], F32)
        ones = big.tile([P, 1], F32)
        ones1 = big.tile([1, P], BF16)
        ones2 = big.tile([P, 2, P], FP8)
        bneg63 = big.tile([P, 1], F32)
        bhalf = big.tile([P, 1], F32)

        nc.sync.dma_start(ohp[:], ohp_d)
        nc.sync.dma_start(ohn[:], ohn_d)
        nc.vector.memset(ones[:], 1.0)
        nc.vector.memset(ones1[:], 1.0)
        nc.vector.memset(ones2[:], 1.0)
        nc.vector.memset(bneg63[:], -63.0)
        nc.vector.memset(bhalf[:], 0.5)

        state = {}

        def prep_load(s):
            """DMA the 8 bf16 chunks of slice s; square into fp8 DR pairs."""
            lts, xsqs = [], []
            for cp in range(NCP):
                x2 = sqp.tile([P, 2, NB], FP8, tag="xsq", name="xsq")
                for r in range(2):
                    c = 2 * cp + r
                    lt = xload.tile([P, NB], BF16, tag="lt", name="lt")
                    nc.sync.dma_start(
                        lt[:], xt_d[c * P:(c + 1) * P, s * NB:(s + 1) * NB])
                    if c < 3:
                        nc.scalar.square(x2[:, r, :], lt[:])
                    elif c < 5:
                        nc.vector.tensor_mul(x2[:, r, :], lt[:], lt[:])
                    else:
                        nc.gpsimd.tensor_mul(x2[:, r, :], lt[:], lt[:])
                    lts.append(lt)
                xsqs.append(x2)
            state[s] = (lts, xsqs)

        def prep_pe(s):
            """Column norms via fp8-DR ones-matmul, inv broadcast, normalize."""
            lts, xsqs = state.pop(s)
            sq_ps = psum2.tile([P, NB], F32, tag="sqps", name="sq_ps")
            for cp in range(NCP):
                nc.tensor.matmul(sq_ps[:], ones2[:], xsqs[cp][:], perf_mode=DR,
                                 start=(cp == 0), stop=(cp == NCP - 1))
            # nrow = sqrt(sq)/XSCALE in bf16, so 1/nrow folds the fp8 scale
            nrow = nrows.tile([1, NB], BF16, tag="nrow", name="nrow")
            nc.scalar.activation(nrow[:], sq_ps[0:1, :], AF.Sqrt,
                                 scale=1.0 / (XSCALE * XSCALE))
            bc_ps = psum2.tile([P, NB], F32, tag="bcps", name="bc_ps")
            nc.tensor.matmul(bc_ps[:], ones1[:], nrow[:], start=True, stop=True)
            invb = invp.tile([P, NB], F32, tag="invb", name="invb")
            nc.vector.reciprocal_approx_fast(invb[:], bc_ps[:])
            for c in range(KC):
                eng = nc.vector if c < 4 else nc.gpsimd
                eng.tensor_mul(xnts[s][:, c, :], lts[c][:], invb[:])

        def reduce_tile(pts, m, s):
            col = m * NJ + s
            rld = scratch.tile([P, NB], F32, tag="rld", name="rld")
            nc.scalar.activation(rld[:], pts[:], AF.Relu,
                                 bias=bneg63[:], scale=-PSC,
                                 accum_out=pos_all[:, col:col + 1])
            nc.vector.reduce_max(max_all[:, col:col + 1], pts[:], axis=AX.X)

        def main_pair(s0, s1):
            """m = 256*(g - 64*same): fp8-DR matmuls, lhsT shared across the
            slice pair so each weight load covers two matmuls."""
            for m in range(NM):
                pa = psum.tile([P, NB], F32, tag="pt", name="pt")
                pb = psum.tile([P, NB], F32, tag="pt", name="pt")
                for cp in range(NCP):
                    lhsT = xnts[0][:, 2 * cp:2 * cp + 2, m * P:(m + 1) * P]
                    nc.tensor.matmul(pa[:], lhsT, xnts[s0][:, 2 * cp:2 * cp + 2, :],
                                     perf_mode=DR, start=(cp == 0), stop=False)
                    nc.tensor.matmul(pb[:], lhsT, xnts[s1][:, 2 * cp:2 * cp + 2, :],
                                     perf_mode=DR, start=(cp == 0), stop=False)
                ohl = ohp[:, m * P:(m + 1) * P]
                nc.tensor.matmul(pa[:], ohl, ohn[:, s0 * NB:(s0 + 1) * NB],
                                 start=False, stop=True)
                nc.tensor.matmul(pb[:], ohl, ohn[:, s1 * NB:(s1 + 1) * NB],
                                 start=False, stop=True)
                reduce_tile(pa, m, s0)
                reduce_tile(pb, m, s1)

        # ---- pipelined schedule: loads 2 pairs ahead, prep-PE 1 pair ahead
        prep_load(0); prep_load(1); prep_load(2); prep_load(3)
        prep_pe(0); prep_pe(1)
        for pr in range(NJ // 2):
            if pr < 2:
                prep_load(2 * pr + 4)
                prep_load(2 * pr + 5)
            if pr < 3:
                prep_pe(2 * pr + 2)
                prep_pe(2 * pr + 3)
            main_pair(2 * pr, 2 * pr + 1)

        # ---- tail: per-anchor loss, partition-sum, scale ----
        posg = small.tile([P, NM], F32)
        nc.vector.reduce_sum(posg[:], pos_all.rearrange("p (m j) -> p m j", j=NJ),
                             axis=AX.X)
        maxg = small.tile([P, NM], F32)
        nc.vector.reduce_max(maxg[:], max_all.rearrange("p (m j) -> p m j", j=NJ),
                             axis=AX.X)
        hneg = small.tile([P, NM], F32)
        nc.scalar.activation(hneg[:], maxg[:], AF.Relu, bias=1.0, scale=-PSC)
        diff = small.tile([P, NM], F32)
        nc.vector.tensor_sub(diff[:], posg[:], hneg[:])
        loss = small.tile([P, NM], F32)
        nc.scalar.activation(loss[:], diff[:], AF.Relu, bias=bhalf[:], scale=2.0)
        psc = psum2.tile([1, NM], F32, tag="psc")
        nc.tensor.matmul(psc[:], ones[:], loss[:], start=True, stop=True)
        red = small.tile([1, 1], F32)
        nc.vector.reduce_sum(red[:], psc[:], axis=AX.X)
        outt = small.tile([1, 1], F32)
        nc.scalar.mul(outt[:], red[:], 1.0 / B)
        nc.sync.dma_start(out_d, outt[:])

    nc.compile()
    return nc


_NC = None


def _get_nc():
    global _NC
    if _NC is None:
        _NC = build_kernel()
    return _NC


def make_in_maps(x, label):
    x = np.asarray(x, dtype=np.float32)
    label = np.asarray(label).astype(np.int64)
    xT = np.ascontiguousarray(x.T.astype(ml_dtypes.bfloat16))
    oh = np.zeros((NLAB, B), dtype=np.float32)
    oh[label, np.arange(B)] = 1.0
    in_maps = []
    for c in range(NCORES):
        k = c * BA
        xTr = np.concatenate([xT[:, k:], xT[:, :k]], axis=1)
        ohr = np.concatenate([oh[:, k:], oh[:, :k]], axis=1)
        in_maps.append({
            "xT": np.ascontiguousarray(xTr),
            "ohp": np.ascontiguousarray(
                (ALPHA * ohr[:, :BA]).astype(ml_dtypes.float8_e4m3)),
            "ohn": np.ascontiguousarray(
                (-ALPHA * ohr).astype(ml_dtypes.float8_e4m3)),
        })
    return in_maps


def kernel(x, label):
    nc = _get_nc()
    res = run_bass_kernel_spmd(nc, make_in_maps(x, label),
                               core_ids=list(range(NCORES)))
    total = sum(float(r["out"][0, 0]) for r in res.results)
    return np.float32(total)
